# revision 118
# baseline (speedup 1.0000x reference)
"""AttentionPooling Trainium2 kernel (v2).

Self-contained: takes full (unsharded) numpy inputs, shards edges across 8
NeuronCores (2 graphs per core), runs a Bass/Tile kernel SPMD, gathers the
per-graph [2, 256] outputs into the full [16, 256] result.

v2 design (vs baseline):
  - Edges + folded QK/V projection matrix in fp8(e4m3, scaled) with
    DoubleRow matmuls: K=256 contraction in one pass at 0.5 cyc/row.
  - Pair-wise etile processing: one [128, 2x256] exp per 2 etiles.
  - Act-table discipline: only Exp/Copy/Identity/Tanh (one table) during the
    main body; single Sqrt table swap (prefetched) after phase 1; silu via
    tanh so no Sigmoid table is ever loaded.
  - MLP: W1 streamed as the moving operand into 4 interleaved PSUM
    accumulation groups (hides the per-matmul SBUF access latency).
  - Software-pipelined emission so the PE never waits on the scalar exp.
  - DMA issues spread across engine queues (sync: edges/G, tensor: W1,
    gpsimd: misc) with critical chunks first.
"""
import math
from contextlib import ExitStack

import numpy as np
import ml_dtypes

import concourse.bass as bass
import concourse.mybir as mybir
import concourse.tile as tile
from concourse import bacc
from concourse.bass_utils import run_bass_kernel_spmd

BF16 = ml_dtypes.bfloat16
F8 = ml_dtypes.float8_e4m3  # TRN fp8e4: max normal 240
N_CORES = 8
NH = 8          # attention heads
LN_EPS = 1e-5
F8_TARGET = 200.0

_NC_CACHE = {}
LAST_RESULT = None


def build_nc(T, NG=2, H=256, S=32, exp_scale=1.0, sv=1.0,
             use_b1=True, use_b2=True, debug=False):
    """Per-core Bass program.

    T  = 128-edge tiles per graph (8), NG = graphs per core (2)
    Column layouts:
      scores cols j = h*S + s (h-major over 8 heads -> 256)
      v      cols j = h*HD + d (h-major -> 256)
    """
    dt = mybir.dt
    AF = mybir.ActivationFunctionType
    DR = mybir.MatmulPerfMode.DoubleRow
    HD = H // NH
    EC = NG * T * 128            # edge columns per core
    NT = NG * T                  # etiles per core
    NP = NT // 2                 # etile pairs

    nc = bacc.Bacc("TRN2")
    edges8 = nc.dram_tensor("edges8", [128, 2, EC], dt.float8e4, kind="ExternalInput")
    gm8 = nc.dram_tensor("gm8", [128, 2, 2 * H], dt.float8e4, kind="ExternalInput")
    w1k = nc.dram_tensor("w1k", [128, 2 * S * H], dt.bfloat16, kind="ExternalInput")
    # wc: wo0' | wo1' | w2k0*0.5 | w2k1*0.5   (wo' = Wo/(s_e*s_g))
    wc = nc.dram_tensor("wc", [128, 4 * H], dt.bfloat16, kind="ExternalInput")
    # sc: seedsb(=seeds+bo) | id32
    sc = nc.dram_tensor("sc", [S, H + S], dt.bfloat16, kind="ExternalInput")
    # br: b1p | b2 | ones(2)  (row vectors)
    br = nc.dram_tensor("br", [1, 2 * H + 4], dt.bfloat16, kind="ExternalInput")
    out = nc.dram_tensor("out", [NG, H], dt.float32, kind="ExternalOutput")
    if debug:
        dbg_numt = nc.dram_tensor("dbg_numt", [128, 2, H], dt.bfloat16,
                                  kind="ExternalOutput")
        dbg_att = nc.dram_tensor("dbg_att", [128, 258], dt.float32,
                                 kind="ExternalOutput")
        dbg_y = nc.dram_tensor("dbg_y", [S, H], dt.float32,
                               kind="ExternalOutput")
        dbg_flat = nc.dram_tensor("dbg_flat", [128, 2, S, NG], dt.bfloat16,
                                  kind="ExternalOutput")
        dbg_pre = nc.dram_tensor("dbg_pre", [NG, H], dt.float32,
                                 kind="ExternalOutput")

    with tile.TileContext(nc) as tc, ExitStack() as ctx:
        _ctr = [0]

        def mk(pool, shape, dtype, tag):
            _ctr[0] += 1
            return pool.tile(shape, dtype, tag=tag, name=f"{tag}_{_ctr[0]}")

        singles = ctx.enter_context(tc.tile_pool(name="singles", bufs=1))
        pnum = ctx.enter_context(tc.tile_pool(name="pnum", bufs=2))
        pva = ctx.enter_context(tc.tile_pool(name="pva", bufs=4))
        pg = ctx.enter_context(tc.tile_pool(name="pg", bufs=2))
        # psY pool stays open across both phases (2 banks); the big phase-1
        # pools live in a nested scope freed before the MLP-phase pools open.
        psXp = ctx.enter_context(tc.tile_pool(name="psX", bufs=2, space="PSUM"))
        phase1 = ExitStack()
        # scores and v live in separate pools so the exp alone frees the
        # scores bank (the v casts drain independently) — shortens the
        # phase-1 software-pipeline cycle
        psSp = phase1.enter_context(tc.tile_pool(name="psS", bufs=2, space="PSUM"))
        psVp = phase1.enter_context(tc.tile_pool(name="psV", bufs=2, space="PSUM"))
        # A and B share one bank; only the very first matmul into the bank
        # carries start=True (first_mm clears has_written for the whole 2KB
        # zero-region, so a second start would wipe the other group's bits).
        psABp = phase1.enter_context(tc.tile_pool(name="psAB", bufs=2, space="PSUM"))

        # ---- DMA issues, all on the sync HWDGE queue in consumption order:
        # the DMA fabric serves transfers roughly in issue order, so the
        # phase-1-critical chunks (G, edges, Wo) must precede the 4MB W1.
        # warm-up operands memset first so the Tensor engine can start its
        # p-state ramp immediately after the framework preamble
        junkw = mk(singles, [128, 8], dt.bfloat16, "junkw")
        junkr = mk(singles, [128, 512], dt.bfloat16, "junkr")
        nc.gpsimd.memset(junkw, 0.0)
        nc.gpsimd.memset(junkr, 0.0)

        sb_g = mk(singles, [128, 2, 2 * H], dt.float8e4, "g8")
        sb_e = mk(singles, [128, 2, EC], dt.float8e4, "e8")
        sb_wc = mk(singles, [128, 4 * H], dt.bfloat16, "wc")
        sb_w1 = mk(singles, [128, 2 * S * H], dt.bfloat16, "w1")
        # The DMA fabric's throughput ramp scales with outstanding transfers,
        # so queue many small edge chunks immediately across both HWDGE
        # queues; W1's 4MB follows once the phase-1-critical data is queued.
        nc.scalar.dma_start(sb_g, gm8[:])
        nc.sync.dma_start(sb_e[:, :, 0:128], edges8[:, :, 0:128])
        nc.scalar.dma_start(sb_e[:, :, 512:1024], edges8[:, :, 512:1024])
        nc.sync.dma_start(sb_e[:, :, 128:512], edges8[:, :, 128:512])
        nc.sync.dma_start(sb_e[:, :, 1024:1536], edges8[:, :, 1024:1536])
        nc.sync.dma_start(sb_e[:, :, 1536:EC], edges8[:, :, 1536:EC])
        nc.sync.dma_start(sb_wc, wc[:])
        W1C = 2 * S * H // 4
        for c in range(4):
            nc.sync.dma_start(sb_w1[:, c * W1C:(c + 1) * W1C],
                              w1k[:, c * W1C:(c + 1) * W1C])

        sb_sc = mk(singles, [S, H + S], dt.bfloat16, "sc")
        nc.gpsimd.dma_start(sb_sc, sc[:])
        sb_br = mk(singles, [1, 2 * H + 4], dt.bfloat16, "br")
        nc.gpsimd.dma_start(sb_br, br[:])

        wo = [sb_wc[:, 0:H], sb_wc[:, H:2 * H]]
        w2 = [sb_wc[:, 2 * H:3 * H], sb_wc[:, 3 * H:4 * H]]
        seedsb = sb_sc[0:S, 0:H]
        id32 = sb_sc[0:S, H:H + S]
        id2 = sb_sc[0:2, H:H + 2]
        b1row = sb_br[0:1, 0:H]
        b2row = sb_br[0:1, H:2 * H]
        ones2 = sb_br[0:1, 2 * H:2 * H + 2]

        sb_eps = mk(singles, [2 * S, 1], dt.float32, "eps")
        nc.gpsimd.memset(sb_eps, LN_EPS)
        # per-partition bias column ln(0.4) for the range-shifted fp8 exp
        sb_shift = mk(singles, [128, 1], dt.float32, "shift")
        nc.gpsimd.memset(sb_shift, -0.916290731874155)

        # ---- PE p-state warm-up: the tensor engine needs ~3us of sustained
        # activity to reach full clock. Junk matmuls fill the otherwise-idle
        # window while the first edge/G DMAs are in flight, so phase 1 starts
        # at speed instead of at the 0.65GHz cold p-state.
        wtile = mk(psSp, [128, 2, H], dt.float32, "psS")
        for w in range(4):
            nc.tensor.matmul(wtile[0:8, w % 2, :], junkw, junkr[:, 0:H],
                             start=True, stop=True, skip_group_check=True)
        sb_flat = mk(singles, [128, 2, S, NG], dt.bfloat16, "flat")
        mv64 = mk(singles, [2 * S, 2], dt.float32, "mv64")
        std64 = mk(singles, [2 * S, 1], dt.float32, "std64")
        rstd64 = mk(singles, [2 * S, 1], dt.float32, "rstd64")

        # ---- phase 1, software pipelined over etile pairs
        psS = [None] * NP
        psV = [None] * NP
        psAB = [None] * NG
        numt = [None] * NP
        gdone = [None] * NG   # per-graph tail state

        def emit_proj(q):
            psS[q] = mk(psSp, [128, 2, H], dt.float32, "psS")
            psV[q] = mk(psVp, [128, 2, H], dt.float32, "psV")
            lhs = [sb_e[:, :, (2 * q + j) * 128:(2 * q + j) * 128 + 128]
                   for j in range(2)]
            # both score matmuls first so the exp unblocks earliest
            for j in range(2):
                nc.tensor.matmul(psS[q][:, j, :], lhs[j], sb_g[:, :, 0:H],
                                 start=True, stop=True, perf_mode=DR,
                                 skip_group_check=True)
            for j in range(2):
                nc.tensor.matmul(psV[q][:, j, :], lhs[j], sb_g[:, :, H:2 * H],
                                 start=True, stop=True, perf_mode=DR,
                                 skip_group_check=True)

        def emit_tail_vecA(g):
            if debug and g == 0:
                adbg = mk(pg, [128, 258], dt.float32, "adbg")
                nc.vector.tensor_copy(adbg, psAB[0])
                nc.sync.dma_start(dbg_att[:], adbg)
            # reciprocals of the two denominator columns
            ra = mk(pg, [128, 1], dt.float32, "ra")
            rb = mk(pg, [128, 1], dt.float32, "rb")
            nc.vector.reciprocal(ra, psAB[g][:, 128:129])
            nc.vector.reciprocal(rb, psAB[g][:, 257:258])
            gdone[g] = dict(ra=ra, rb=rb)

        def emit_tail_vecB(g):
            st = gdone[g]
            # full-width normalize: 2 column-priced DVE ops replace 8 block
            # muls; the per-head extraction happens in the (cheap) transposes
            attca = mk(pg, [128, 128], dt.bfloat16, "attca")
            attcb = mk(pg, [128, 128], dt.bfloat16, "attcb")
            nc.vector.tensor_scalar_mul(attca, psAB[g][:, 0:128], st["ra"])
            nc.vector.tensor_scalar_mul(attcb, psAB[g][:, 129:257], st["rb"])
            st["attca"], st["attcb"] = attca, attcb

        def emit_tail_C(g):
            st = gdone[g]
            # per-head 32x32 transposes gather the diagonal blocks into the
            # stacked [128, 32] layout psY's lhsT needs (all-regular APs)
            attTa = mk(pg, [128, S], dt.bfloat16, "attTa")
            attTb = mk(pg, [128, S], dt.bfloat16, "attTb")
            for h in range(4):
                blk = slice(32 * h, 32 * h + 32)
                nc.vector.transpose(attTa[blk, :], st["attca"][blk, blk])
                nc.vector.transpose(attTb[blk, :], st["attcb"][blk, blk])
            psY = mk(psXp, [S, H], dt.float32, "psY")
            # seedsb has no attention dependency: run it first so the psum
            # bank is claimed the moment the PE is free
            nc.tensor.matmul(psY, id32, seedsb, start=True, stop=False,
                             skip_group_check=True)
            nc.tensor.matmul(psY, attTa, wo[0], start=False, stop=False,
                             skip_group_check=True)
            nc.tensor.matmul(psY, attTb, wo[1], start=False, stop=True,
                             skip_group_check=True)
            st6 = mk(pg, [S, 6], dt.float32, "st6")
            nc.vector.bn_stats(st6, psY)
            nc.vector.bn_aggr(mv64[g * S:(g + 1) * S, :], st6)
            st["psY"] = psY

        def emit_rest(q):
            PG = T // 2  # pairs per graph
            g, qg = q // PG, q % PG
            # exp over both etiles' score halves, fp8 output scaled by 0.4
            # (bias=ln(0.4) keeps exp within fp8e4 range; the scale cancels
            # in the softmax ratio)
            numt[q] = mk(pnum, [128, 2, H], dt.bfloat16, "numt")
            nc.scalar.activation(numt[q], psS[q], AF.Exp,
                                 scale=float(exp_scale), bias=sb_shift)
            # v halves rescaled (scale folds into Wo on the host)
            vaA = mk(pva, [128, 2, 129], dt.bfloat16, "vaA")
            vaB = mk(pva, [128, 2, 129], dt.bfloat16, "vaB")
            nc.gpsimd.memset(vaA[:, :, 128:129], 1.0)
            nc.gpsimd.memset(vaB[:, :, 128:129], 1.0)
            nc.vector.tensor_scalar_mul(vaA[:, :, 0:128],
                                        psV[q][:, :, 0:128], float(sv))
            nc.vector.tensor_scalar_mul(vaB[:, :, 0:128],
                                        psV[q][:, :, 128:H], float(sv))
            if qg == 0:
                psAB[g] = mk(psABp, [128, 258], dt.float32, "psAB")
            for j in range(2):
                t = (2 * q + j) % T
                # single bank: only the very first matmul carries start=True
                nc.tensor.matmul(psAB[g][:, 0:129], numt[q][:, j, 0:128],
                                 vaA[:, j, :], start=(t == 0),
                                 stop=(t == T - 1), skip_group_check=True)
                nc.tensor.matmul(psAB[g][:, 129:258], numt[q][:, j, 128:256],
                                 vaB[:, j, :], start=False,
                                 stop=(t == T - 1), skip_group_check=True)
            if debug and q == 0:
                nc.sync.dma_start(dbg_numt[:], numt[0])
            # graph-0 tail work is spread across later pairs' engine slack
            if q == PG:
                emit_tail_vecA(0)
            elif q == PG + 1:
                emit_tail_vecB(0)
            elif q == PG + 2:
                emit_tail_C(0)

        emit_proj(0)
        emit_proj(1)
        for q in range(NP):
            emit_rest(q)
            if q + 2 < NP:
                emit_proj(q + 2)

        # keep the PE p-state up through the tail window: junk matmuls fill
        # the gap between the last attention matmul and graph-1's psY
        wtail = mk(psSp, [128, 2, H], dt.float32, "psS")
        for w in range(6):
            nc.tensor.matmul(wtail[0:8, w % 2, :], junkw, junkr[:, 0:H],
                             start=True, stop=True, skip_group_check=True)

        # prefetch the sqrt act table during graph-1's vector tail work; the
        # dummy reads numt of the last pair so it cannot be hoisted earlier.
        sdum = mk(pg, [1, 1], dt.float32, "sdum")
        nc.scalar.activation(sdum, numt[NP - 1][0:1, 0, 0:1], AF.Sqrt)

        emit_tail_vecA(1)
        emit_tail_vecB(1)
        # all psPP/psAB reads are emitted; free their banks for the MLP pools
        phase1.close()
        psMp = ctx.enter_context(tc.tile_pool(name="psM", bufs=1, space="PSUM"))
        psFin = ctx.enter_context(tc.tile_pool(name="psFin", bufs=2, space="PSUM"))
        psFin1 = ctx.enter_context(tc.tile_pool(name="psFin1", bufs=1, space="PSUM"))

        # ---- LN finish per graph (sqrt table was prefetched by sdum, so
        # each graph's chain runs as soon as its bn stats are ready)
        def emit_ln(g):
            sl = slice(g * S, (g + 1) * S)
            nc.scalar.activation(std64[sl, :], mv64[sl, 1:2], AF.Sqrt,
                                 bias=sb_eps[sl, :])
            nc.vector.reciprocal(rstd64[sl, :], std64[sl, :])
            zb = mk(pg, [S, H], dt.bfloat16, "zb")
            # z = (y - mu) * rstd, halves split across vector and scalar
            # (Identity is in the resident sqrt table; scalar is idle here)
            tbn = mk(pg, [S, 1], dt.float32, "tbn")
            nc.vector.scalar_tensor_tensor(tbn, mv64[sl, 0:1], -1.0,
                                           rstd64[sl, :],
                                           mybir.AluOpType.mult,
                                           mybir.AluOpType.mult)
            nc.vector.tensor_scalar(zb[:, 0:128], gdone[g]["psY"][:, 0:128],
                                    mv64[sl, 0:1], rstd64[sl, :],
                                    mybir.AluOpType.subtract,
                                    mybir.AluOpType.mult)
            nc.scalar.activation(zb[:, 128:256], gdone[g]["psY"][:, 128:256],
                                 AF.Identity, bias=tbn, scale=rstd64[sl, :])
            for half in range(2):
                psZ = mk(psFin, [128, S], dt.bfloat16, "psZ")
                nc.tensor.transpose(psZ, zb[:, half * 128:(half + 1) * 128], id32)
                # scalar moves flat slices so the DVE stays free for the MLP
                nc.scalar.activation(sb_flat[:, half, :, g], psZ, AF.Copy)

        # graph 0's LN/flat work fills the PE + vector gap while graph 1's
        # attention normalize chain resolves
        emit_ln(0)
        emit_tail_C(1)
        emit_ln(1)
        # restore the exp-family table (for the final Tanh) during the MLP:
        # Tanh forces exp_and_others; reading std64 pins it after the sqrts.
        cdum = mk(pg, [1, 1], dt.float32, "cdum")
        nc.scalar.activation(cdum, std64[S:S + 1, :], AF.Tanh)

        # ---- MLP: pre1 = flat @ W1g + b1p, into 4 interleaved psum groups
        psM = mk(psMp, [128, H], dt.float32, "psM4")
        # a few more PE keep-warm matmuls while zb/flat for graph 1 resolve
        for w in range(3):
            nc.tensor.matmul(psM[64:72, :], junkw, junkr[:, 0:H],
                             start=True, stop=True, skip_group_check=True)
        NK = 2 * S  # 64 k-tiles over 3 interleaved accumulation groups
        for kt in range(NK):
            j = kt % 3
            half = kt & 1
            s = kt >> 1
            nc.tensor.matmul(psM[32 * j:32 * j + NG, :], sb_flat[:, half, s, :],
                             sb_w1[:, kt * H:(kt + 1) * H],
                             start=(kt < 3),
                             stop=(kt >= NK - 3 and (j != 0 or not use_b1)),
                             tile_position=(0, 32 * j),
                             skip_group_check=True)
        if use_b1:
            nc.tensor.matmul(psM[0:NG, :], ones2, b1row, start=False, stop=True,
                             skip_group_check=True)
        if debug:
            nc.sync.dma_start(dbg_flat[:], sb_flat)
        # combine the accumulators; DVE ops allow at most one PSUM operand,
        # so stage the first quarter into SBUF via the scalar engine
        s0 = mk(pg, [NG, H], dt.float32, "s0")
        nc.scalar.activation(s0, psM[0:NG, :], AF.Copy)
        c01 = mk(pg, [NG, H], dt.float32, "c01")
        pre = mk(pg, [NG, H], dt.float32, "pre")
        nc.vector.tensor_add(c01, s0, psM[32:32 + NG, :])
        nc.vector.tensor_add(pre, c01, psM[64:64 + NG, :])
        # silu via tanh (exp-family table; Sigmoid would force a table swap):
        # 2*silu(x) = x*(1+tanh(x/2)); the 0.5 is folded into W2.
        if debug:
            nc.sync.dma_start(dbg_pre[:], pre)
        th = mk(pg, [NG, H], dt.float32, "th")
        nc.scalar.activation(th, pre, AF.Tanh, scale=0.5)
        # 2*silu(pre) = (tanh(pre/2) + 1) * pre, fused in one DVE op
        h1b = mk(pg, [NG, H], dt.bfloat16, "h1b")
        nc.vector.scalar_tensor_tensor(h1b, th, 1.0, pre,
                                       mybir.AluOpType.add,
                                       mybir.AluOpType.mult)
        h1T = []
        for half in range(2):
            psT = mk(psFin1, [128, NG], dt.bfloat16, "psT")
            nc.tensor.transpose(psT, h1b[:, half * 128:(half + 1) * 128], id2)
            h1Tk = mk(pg, [128, NG], dt.bfloat16, f"h1T{half}")
            nc.vector.tensor_copy(h1Tk, psT)
            h1T.append(h1Tk)
        psO = mk(psFin1, [NG, H], dt.float32, "psO")
        nc.tensor.matmul(psO, h1T[0], w2[0], start=True, stop=False,
                         skip_group_check=True)
        nc.tensor.matmul(psO, h1T[1], w2[1], start=False, stop=not use_b2,
                         skip_group_check=True)
        if use_b2:
            nc.tensor.matmul(psO, ones2, b2row, start=False, stop=True,
                             skip_group_check=True)
        outsb = mk(pg, [NG, H], dt.float32, "outsb")
        nc.vector.tensor_copy(outsb, psO)
        nc.sync.dma_start(out[:], outsb)

    nc.compile()
    return nc


def host_prep(inputs):
    """Host-side preprocessing: fold weights, quantize, shard edges."""
    ef = np.asarray(inputs["edge_features"], np.float32)
    batch = np.asarray(inputs["batch"], np.int64)
    seeds = np.asarray(inputs["seed_vectors"], np.float32)
    Wq = np.asarray(inputs["Wq"], np.float32)
    Wk = np.asarray(inputs["Wk"], np.float32)
    Wv = np.asarray(inputs["Wv"], np.float32)
    Wo = np.asarray(inputs["Wo"], np.float32)
    bo = np.asarray(inputs["bo"], np.float32)
    ln_g = np.asarray(inputs["ln_g"], np.float32)
    ln_b = np.asarray(inputs["ln_b"], np.float32)
    W1 = np.asarray(inputs["W1"], np.float32)
    b1 = np.asarray(inputs["b1"], np.float32)
    W2 = np.asarray(inputs["W2"], np.float32)
    b2 = np.asarray(inputs["b2"], np.float32)
    B = int(np.asarray(inputs["num_graphs"]))

    E, H = ef.shape
    S = seeds.shape[0]
    HD = H // NH
    NG = B // N_CORES
    CNT = E // B                     # edges per graph (equal segments)
    T = CNT // 128
    EC = NG * T * 128

    # folded score/v projection matrix G = [qk | Wv], [H, 2H]
    q = seeds @ Wq
    qk = np.einsum("chd,shd->chs",
                   Wk.reshape(H, NH, HD),
                   q.reshape(S, NH, HD)).reshape(H, NH * S)
    qk *= 1.0 / np.sqrt(HD)
    G = np.concatenate([qk, Wv], axis=1)                  # [H, 2H]

    s_e = F8_TARGET / max(np.abs(ef).max(), 1e-20)
    s_g = F8_TARGET / max(np.abs(G).max(), 1e-20)
    # safe upper bound on |v| = |ef @ Wv| for the on-device fp8 quantization
    # of the v columns (fp8 is scale-invariant, only overflow matters)
    vbound = float(np.linalg.norm(ef, axis=1).max()
                   * np.linalg.norm(Wv, axis=0).max())
    s_v = F8_TARGET / (vbound * s_e * s_g)

    def q8(x, s):
        return np.asarray(np.clip(x * s, -240.0, 240.0), F8)

    # G packed for DoubleRow: [p, i, o] = G[i*128+p, o]
    gm8 = np.ascontiguousarray(
        q8(G, s_g).astype(np.float32).reshape(2, 128, 2 * H)
        .transpose(1, 0, 2))
    gm8 = np.asarray(gm8, F8)

    seedsb = seeds + bo[None, :]
    W1g = (W1.reshape(S, H, H) * ln_g[None, :, None]).reshape(S * H, H)
    b1p = b1 + ln_b @ W1.reshape(S, H, H).sum(axis=0)

    wc = np.zeros((128, 4 * H), np.float32)
    wo_f = Wo / (s_e * s_g * s_v)
    wc[:, 0:H] = wo_f[0:128]
    wc[:, H:2 * H] = wo_f[128:256]
    wc[:, 2 * H:3 * H] = 0.5 * W2[0:128]
    wc[:, 3 * H:4 * H] = 0.5 * W2[128:256]

    sc = np.zeros((S, H + S), np.float32)
    sc[:, 0:H] = seedsb
    sc[:, H:H + S] = np.eye(S, dtype=np.float32)

    br = np.zeros((1, 2 * H + 4), np.float32)
    br[0, 0:H] = b1p
    br[0, H:2 * H] = b2
    br[0, 2 * H:2 * H + 2] = 1.0

    common = {
        "gm8": gm8,
        "wc": wc.astype(BF16),
        "sc": sc.astype(BF16),
        "br": br.astype(BF16),
        "w1k": np.ascontiguousarray(
            W1g.reshape(2 * S, 128, H).transpose(1, 0, 2).reshape(
                128, 2 * S * H)).astype(BF16),
    }

    in_maps = []
    for core in range(N_CORES):
        # edges for this core's NG graphs, [128, 2, EC] DoubleRow packing
        eT = ef[core * NG * CNT:(core + 1) * NG * CNT].T    # [H, EC]
        e8 = q8(eT, s_e).astype(np.float32).reshape(2, 128, EC)
        m = dict(common)
        m["edges8"] = np.asarray(
            np.ascontiguousarray(e8.transpose(1, 0, 2)), F8)
        in_maps.append(m)
    flags = (bool(np.any(b1p)), bool(np.any(b2)))
    return in_maps, T, NG, 1.0 / (s_e * s_g), s_v, flags


def _pattern_ok(inputs):
    try:
        batch = np.asarray(inputs["batch"], np.int64)
        B = int(np.asarray(inputs["num_graphs"]))
        ef = np.asarray(inputs["edge_features"])
        seeds = np.asarray(inputs["seed_vectors"])
        E = ef.shape[0]
        if not (B % N_CORES == 0 and B > 0 and E % B == 0
                and (E // B) % 128 == 0
                and ef.ndim == 2 and ef.shape[1] == 256
                and seeds.shape == (32, 256)):
            return False
        # equal, sorted segments required (no masking path in the kernel)
        expect = np.repeat(np.arange(B, dtype=np.int64), E // B)
        return bool(np.array_equal(batch, expect))
    except Exception:
        return False


def _numpy_reference(inputs):
    """Pure-numpy fallback matching the reference semantics."""
    ef = np.asarray(inputs["edge_features"], np.float64)
    batch = np.asarray(inputs["batch"], np.int64)
    seeds = np.asarray(inputs["seed_vectors"], np.float64)
    Wq, Wk, Wv, Wo = (np.asarray(inputs[k], np.float64)
                      for k in ("Wq", "Wk", "Wv", "Wo"))
    bo, ln_g, ln_b = (np.asarray(inputs[k], np.float64)
                      for k in ("bo", "ln_g", "ln_b"))
    W1, b1, W2, b2 = (np.asarray(inputs[k], np.float64)
                      for k in ("W1", "b1", "W2", "b2"))
    B = int(np.asarray(inputs["num_graphs"]))
    S, H = seeds.shape
    hd = H // NH
    q = (seeds @ Wq).reshape(S, NH, hd)
    k = (ef @ Wk).reshape(-1, NH, hd)
    v = (ef @ Wv).reshape(-1, NH, hd)
    scores = np.einsum("shd,ehd->esh", q, k) / np.sqrt(hd)
    out = np.zeros((B, S, NH, hd))
    for b in range(B):
        m = batch == b
        s = scores[m]
        s = s - s.max(axis=0, keepdims=True)
        w = np.exp(s)
        w /= w.sum(axis=0, keepdims=True)
        out[b] = np.einsum("esh,ehd->shd", w, v[m])
    att = out.reshape(B, S, H)
    y = seeds[None] + att @ Wo + bo
    mu = y.mean(-1, keepdims=True)
    var = ((y - mu) ** 2).mean(-1, keepdims=True)
    y = (y - mu) / np.sqrt(var + LN_EPS) * ln_g + ln_b
    flat = y.reshape(B, S * H)
    h1 = flat @ W1 + b1
    h1 = h1 / (1 + np.exp(-h1))
    return (h1 @ W2 + b2).astype(np.float32)


def kernel(**inputs):
    if not _pattern_ok(inputs):
        return _numpy_reference(inputs)
    in_maps, T, NG, exp_scale, s_v, (use_b1, use_b2) = host_prep(inputs)
    key = (T, NG, round(float(exp_scale), 14), round(float(s_v), 10),
           use_b1, use_b2)
    if key not in _NC_CACHE:
        _NC_CACHE[key] = build_nc(T, NG, exp_scale=exp_scale, sv=s_v,
                                  use_b1=use_b1, use_b2=use_b2)
    nc = _NC_CACHE[key]
    res = run_bass_kernel_spmd(nc, in_maps, core_ids=list(range(N_CORES)))
    global LAST_RESULT
    LAST_RESULT = res
    return np.concatenate([res.results[i]["out"] for i in range(N_CORES)],
                          axis=0).astype(np.float32)


if __name__ == "__main__":
    import reference
    inputs = {k: np.asarray(v) for k, v in reference.setup_inputs().items()}
    got = kernel(**inputs)
    want = np.asarray(reference.reference(**reference.setup_inputs()))
    rel = np.abs(got - want).max() / np.abs(want).max()
    print("Relative error:", rel)
